# revision 3
# baseline (speedup 1.0000x reference)
"""Trainium2 Bass kernel for nn_NodeNet (GNN message passing + 15-qubit circuit).

v3: latency-optimized. See kernel2_v2_backup.py docstring for the structural
derivation. Changes vs v2:
- one [128,1024] PSUM tile + single DVE copy per transposed matrix
- m0..m3 merge_cnots batched into 2 wide DVE ops via strided CS views
- combine coefficients replaced by two extra angle columns 2*(th29 +/- thf)
  (trig product identities); signs/scales folded into accumulator scalars
- A/B pairs accumulated directly from m5n (bit0-masked strided views)
- D6/C6 via (lo-hi)(lo+hi) split between Pool and DVE
- emission order == intended execution order for the Tile scheduler
"""

import math

import numpy as np

N_CORES = 8
PI = math.pi
S4 = 1.0 / (4.0 * PI)   # t = angle/(4*pi) + OFF  (per-2pi units)
S2 = 1.0 / (2.0 * PI)   # for double-angle columns
OFF = 16.0

# ---------------- column maps ----------------
# ANG/CS are [128, 2*NA]: cos-t in [0:NA), sin-t in [NA:2*NA).
A0, A1p, A2p, A3, A4 = 0, 1, 2, 3, 4
A5p, A6p, A7 = 5, 6, 7
A1m, A2m, A5m, A6m = 8, 9, 10, 11
Xa10p, Xa10m, Xa11, Xa13, Xd14 = 12, 13, 14, 15, 16
Tt19, Tt20, Tt25, Tt2427, Ttf2, Tt29, Tt30 = 17, 18, 19, 20, 21, 22, 23
Tsum, Tdif = 24, 25     # 2*(th29+thf), 2*(th29-thf); thf = th23+th26
NA = 28                 # cols 26,27 unused (pad for (a u w) view splits)

# coef columns
T19, T20, T25, T2427 = 0, 1, 2, 3
NT19, NT20, NT25, NT2427 = 4, 5, 6, 7
CQ1, CQ2, CMPG, CG = 8, 9, 10, 11
CT1, CT2 = 12, 13
NCOEF = 16

# zacc columns
ZA = 4      # [4:6]  = [-2*A0, -2*A1]
ZB = 6      # [6:8]  = [B0, -B1]
ZUB, ZVA = 8, 9
ZWB, ZWA = 12, 14       # pairs [12:14], [14:16]
ZD6, ZC6 = 18, 19
ZD8a, ZD8b, ZD8, ZC8 = 20, 21, 22, 23
ZZ5, ZT1, ZT2, ZZD = 24, 26, 27, 28
NZ = 30

N_WARM = 24

_cache = {}


def _build_program():
    import concourse.bacc as bacc
    import concourse.mybir as mybir
    import concourse.tile as tile
    from concourse.masks import make_identity

    f32 = mybir.dt.float32
    i32 = mybir.dt.int32
    bf16 = mybir.dt.bfloat16
    Alu = mybir.AluOpType
    Act = mybir.ActivationFunctionType

    nc = bacc.Bacc(
        "TRN2",
        target_bir_lowering=False,
        debug=False,
        enable_asserts=False,
        num_devices=1,
    )

    Ri_d = nc.dram_tensor("Rib", [128, 1024], bf16, kind="ExternalInput").ap()
    Ro_d = nc.dram_tensor("Rob", [128, 1024], bf16, kind="ExternalInput").ap()
    P1_d = nc.dram_tensor("P1", [128, 44], f32, kind="ExternalInput").ap()
    P2_d = nc.dram_tensor("P2", [128, 13], bf16, kind="ExternalInput").ap()
    out_d = nc.dram_tensor("out", [128, 2], f32, kind="ExternalOutput").ap()

    with tile.TileContext(nc) as tc:
        with (
            tc.tile_pool(name="sbuf", bufs=1) as sb,
            tc.tile_pool(name="psd", bufs=1, space="PSUM") as psd,
            tc.tile_pool(name="pstp", bufs=2, space="PSUM") as pstp,
            tc.tile_pool(name="psbb", bufs=1, space="PSUM") as psbb,
            tc.tile_pool(name="psmi", bufs=1, space="PSUM") as psmi,
        ):
            # ---------------- DMAs ----------------
            Ri_sb = sb.tile([128, 1024], bf16, tag="Ri")
            Ro_sb = sb.tile([128, 1024], bf16, tag="Ro")
            P1_sb = sb.tile([128, 44], f32, tag="P1")
            P2_sb = sb.tile([128, 13], bf16, tag="P2")
            nc.sync.dma_start(P1_sb[:], P1_d)
            nc.gpsimd.dma_start(Ri_sb[:], Ri_d)
            nc.sync.dma_start(Ro_sb[:], Ro_d)
            nc.sync.dma_start(P2_sb[:], P2_d)

            TH = P1_sb[:, 13:44]

            def thc(i):
                return P1_sb[:, 13 + i:14 + i]

            # ---------------- constants / warm ----------------
            idb = sb.tile([128, 128], bf16, tag="idb")
            make_identity(nc, idb[:])
            piT = sb.tile([128, 1], f32, tag="piT")
            nc.gpsimd.memset(piT[:], PI)
            mpi = sb.tile([128, 1], f32, tag="mpi")
            nc.gpsimd.memset(mpi[:], -PI)
            p2T = sb.tile([128, 1], f32, tag="p2T")
            nc.gpsimd.memset(p2T[:], 2.0 * PI)
            ones1 = sb.tile([1, 128], f32, tag="ones1")
            nc.gpsimd.memset(ones1[:], 1.0)
            warm = sb.tile([1, 1], f32, tag="warm")
            nc.vector.memset(warm[:], 0.0)
            nc.scalar.activation(warm[:], warm[:], Act.Sin)

            zer = sb.tile([128, 128], bf16, tag="zer")
            nc.vector.memset(zer[:], 0.0)
            dummy = psd.tile([128, 128], f32, tag="dummy")
            for _ in range(N_WARM):
                nc.tensor.matmul(dummy[:], zer[:], zer[:],
                                 start=True, stop=True)

            # ---------------- tiles ----------------
            ANG = sb.tile([128, 2 * NA], f32, tag="ANG")
            KI = sb.tile([128, 2 * NA], i32, tag="KI")
            KF = sb.tile([128, 2 * NA], f32, tag="KF")
            WT = sb.tile([128, 2 * NA], f32, tag="WT")
            CS = sb.tile([128, 2 * NA], f32, tag="CS")
            THCr = sb.tile([1, 12], f32, tag="THCr")
            THCX = sb.tile([128, 5], f32, tag="THCX")
            scr = sb.tile([128, 16], f32, tag="scr")
            coef = sb.tile([128, NCOEF], f32, tag="coef")
            zacc = sb.tile([128, NZ], f32, tag="zacc")
            e2 = sb.tile([128, 8], f32, tag="e2")
            bow = sb.tile([128, 80], bf16, tag="bow")
            RiT = sb.tile([128, 1024], bf16, tag="RiT")
            RoT = sb.tile([128, 1024], bf16, tag="RoT")
            out_sb = sb.tile([128, 2], f32, tag="out")

            M03 = sb.tile([128, 16], f32, tag="M03")   # m0|m1|m2|m3
            m0n = sb.tile([128, 4], f32, tag="m0n")
            m4 = sb.tile([128, 4], f32, tag="m4")
            m5 = sb.tile([128, 16], f32, tag="m5")
            m5n = sb.tile([128, 24], f32, tag="m5n")   # [0:16] m5n, [16:24] -hi
            m6 = sb.tile([128, 16], f32, tag="m6")
            m6n = sb.tile([128, 16], f32, tag="m6n")
            m8 = sb.tile([128, 8], f32, tag="m8")
            m8n = sb.tile([128, 8], f32, tag="m8n")
            uw6 = sb.tile([128, 16], f32, tag="uw6")   # [0:8]=lo-hi, [8:16]=lo+hi
            bb = psbb.tile([128, 80], f32, tag="bb")
            sq = sb.tile([128, 8], f32, tag="sq")
            sq2 = sb.tile([128, 8], f32, tag="sq2")
            sqP = sb.tile([128, 8], f32, tag="sqP")
            sqP2 = sb.tile([128, 8], f32, tag="sqP2")

            V = nc.vector
            P = nc.gpsimd
            S = nc.scalar

            def sn(c):
                return NA + c

            def cspair(c):
                return CS[:].rearrange("p (a c) -> p a c", a=2)[:, :, c]

            def swpair(c):
                return CS[:].rearrange("p (a c) -> p a c", a=2)[:, ::-1, c]

            def csc(c):
                return CS[:, c:c + 1]

            def css(c):
                return CS[:, NA + c:NA + c + 1]

            def bc(v, shape):
                return v.to_broadcast(shape)

            def cf(c):
                return coef[:, c:c + 1]

            def zc(c):
                return zacc[:, c:c + 1]

            # ============ EARLY PHASE (emission order = intent) ============
            # theta t-cols (sin half)
            V.tensor_scalar(ANG[:, sn(Tt19):sn(Tt19) + 2], TH[:, 19:21],
                            S4, OFF, Alu.mult, Alu.add)
            V.tensor_scalar(ANG[:, sn(Tt25):sn(Tt25) + 1], thc(25),
                            S4, OFF, Alu.mult, Alu.add)
            V.tensor_tensor(scr[:, 0:1], thc(24), thc(27), Alu.add)
            V.tensor_scalar(ANG[:, sn(Tt2427):sn(Tt2427) + 1], scr[:, 0:1],
                            S4, OFF, Alu.mult, Alu.add)
            V.tensor_tensor(scr[:, 1:2], thc(23), thc(26), Alu.add)
            V.tensor_scalar(ANG[:, sn(Ttf2):sn(Ttf2) + 1], scr[:, 1:2],
                            S2, OFF, Alu.mult, Alu.add)
            V.tensor_scalar(ANG[:, sn(Tt29):sn(Tt29) + 1], thc(29),
                            S2, OFF, Alu.mult, Alu.add)
            V.tensor_scalar(ANG[:, sn(Tt30):sn(Tt30) + 1], thc(30),
                            S2, OFF, Alu.mult, Alu.add)
            # Tsum/Tdif: 2*(th29 +/- thf)
            V.tensor_tensor(scr[:, 2:3], thc(29), scr[:, 1:2], Alu.add)
            V.tensor_scalar(ANG[:, sn(Tsum):sn(Tsum) + 1], scr[:, 2:3],
                            S2, OFF, Alu.mult, Alu.add)
            V.tensor_tensor(scr[:, 3:4], thc(29), scr[:, 1:2], Alu.subtract)
            V.tensor_scalar(ANG[:, sn(Tdif):sn(Tdif) + 1], scr[:, 3:4],
                            S2, OFF, Alu.mult, Alu.add)

            # THCX (X-block theta sums) — ACT adds
            S.activation(scr[:, 4:5], thc(17), Act.Identity, bias=thc(21))
            S.activation(THCX[:, 0:1], thc(10), Act.Identity, bias=scr[:, 4:5])
            V.tensor_tensor(THCX[:, 1:2], thc(10), scr[:, 4:5], Alu.subtract)
            S.activation(scr[:, 5:6], thc(13), Act.Identity, bias=thc(18))
            S.activation(scr[:, 6:7], scr[:, 5:6], Act.Identity, bias=thc(22))
            S.activation(scr[:, 7:8], thc(14), Act.Identity, bias=thc(19))
            S.activation(scr[:, 8:9], scr[:, 7:8], Act.Identity, bias=thc(28))
            V.tensor_copy(THCX[:, 2:3], thc(11))
            V.tensor_copy(THCX[:, 3:4], scr[:, 6:7])
            V.tensor_scalar(THCX[:, 0:4], THCX[:, 0:4], S4, OFF,
                            Alu.mult, Alu.add)
            V.tensor_scalar(THCX[:, 4:5], scr[:, 8:9], S2, OFF,
                            Alu.mult, Alu.add)

            # X-block t-cols
            XH = scr[:, 9:13]
            V.tensor_scalar(XH, P1_sb[:, 0:4], S4, None, Alu.mult)
            V.tensor_scalar(scr[:, 13:14], P1_sb[:, 4:5], S2, None, Alu.mult)
            V.tensor_tensor(ANG[:, sn(Xa10p):sn(Xa10p) + 2],
                            bc(XH[:, 0:1], (128, 2)), THCX[:, 0:2], Alu.add)
            V.tensor_tensor(ANG[:, sn(Xa11):sn(Xa11) + 2],
                            XH[:, 1:4:2], THCX[:, 2:4], Alu.add)
            V.tensor_tensor(ANG[:, sn(Xd14):sn(Xd14) + 1],
                            scr[:, 13:14], THCX[:, 4:5], Alu.add)

            # THCr (critical theta sums, single-partition row for matmul)
            def thr(i):
                return P1_sb[0:1, 13 + i:14 + i]

            V.tensor_copy(THCr[:, 0:1], thr(0))
            V.tensor_copy(THCr[:, 3:5], P1_sb[0:1, 16:18])
            V.tensor_copy(THCr[:, 7:8], thr(7))
            S.activation(THCr[:, 1:2], thr(1), Act.Identity, bias=thr(15))
            S.activation(THCr[:, 2:3], thr(2), Act.Identity, bias=thr(16))
            S.activation(THCr[:, 5:6], thr(5), Act.Identity, bias=thr(14))
            S.activation(THCr[:, 6:7], thr(6), Act.Identity, bias=thr(15))
            V.tensor_tensor(THCr[:, 8:10], P1_sb[0:1, 14:16],
                            P1_sb[0:1, 28:30], Alu.subtract)
            V.tensor_tensor(THCr[:, 10:12], P1_sb[0:1, 18:20],
                            P1_sb[0:1, 27:29], Alu.subtract)
            V.tensor_scalar(THCr[:], THCr[:], S4, OFF, Alu.mult, Alu.add)

            # ---------------- sincos pipeline -----------------------------
            def sincos(lo, hi):
                V.tensor_scalar(ANG[:, lo:hi], ANG[:, NA + lo:NA + hi],
                                0.25, None, Alu.add)
                av = ANG[:].rearrange("p (a c) -> p a c", a=2)[:, :, lo:hi]
                kiv = KI[:].rearrange("p (a c) -> p a c", a=2)[:, :, lo:hi]
                kfv = KF[:].rearrange("p (a c) -> p a c", a=2)[:, :, lo:hi]
                wv = WT[:].rearrange("p (a c) -> p a c", a=2)[:, :, lo:hi]
                cv = CS[:].rearrange("p (a c) -> p a c", a=2)[:, :, lo:hi]
                V.tensor_copy(kiv, av)
                V.tensor_copy(kfv, kiv)
                V.tensor_tensor(wv, av, kfv, Alu.subtract)
                S.activation(cv, wv, Act.Sin, scale=2.0 * PI)

            sincos(12, 26)

            # ---------------- transposes + matmuls ------------------------
            tpi = pstp.tile([128, 1024], bf16, tag="tp", name="tpi")
            for c in range(8):
                nc.tensor.transpose(tpi[:, c * 128:(c + 1) * 128],
                                    Ri_sb[:, c * 128:(c + 1) * 128], idb[:])
            S.copy(RiT[:], tpi[:])

            tpo = pstp.tile([128, 1024], bf16, tag="tp", name="tpo")
            for c in range(8):
                nc.tensor.transpose(tpo[:, c * 128:(c + 1) * 128],
                                    Ro_sb[:, c * 128:(c + 1) * 128], idb[:])

            Xbf = P2_sb[:, 0:5]
            for c in range(8):
                nc.tensor.matmul(bb[:, c * 10:c * 10 + 5],
                                 Ro_sb[:, c * 128:(c + 1) * 128], Xbf,
                                 start=True, stop=True)
                nc.tensor.matmul(bb[:, c * 10 + 5:c * 10 + 10],
                                 Ri_sb[:, c * 128:(c + 1) * 128], Xbf,
                                 start=True, stop=True)

            V.tensor_scalar(e2[:], P2_sb[:, 5:13], S4, None, Alu.mult)
            bbv = bb[:].rearrange("p (c j) -> p c j", j=10)
            e2v = e2[:].rearrange("p (c o) -> p c o", o=1).to_broadcast(
                (128, 8, 10))
            bowv = bow[:].rearrange("p (c j) -> p c j", j=10)
            V.tensor_tensor(bowv, bbv, e2v, Alu.mult)
            V.tensor_copy(RoT[:], tpo[:])

            # ---------------- coefs: tans on DVE, products on Pool --------
            V.reciprocal(coef[:, CT1:CT1 + 4], CS[:, Tt19:Tt19 + 4])
            V.tensor_tensor(coef[:, T19:T19 + 4],
                            CS[:, NA + Tt19:NA + Tt19 + 4],
                            coef[:, CT1:CT1 + 4], Alu.mult)
            V.tensor_scalar(coef[:, NT19:NT19 + 4], coef[:, T19:T19 + 4],
                            -1.0, None, Alu.mult)
            # G = (c19*c20*c25)^2, CMPG = -pi*G  (Pool TT chain)
            P.tensor_tensor(coef[:, CT1:CT1 + 1], csc(Tt19), csc(Tt20),
                            Alu.mult)
            P.tensor_tensor(coef[:, CT2:CT2 + 1], coef[:, CT1:CT1 + 1],
                            csc(Tt25), Alu.mult)
            P.tensor_tensor(coef[:, CG:CG + 1], coef[:, CT2:CT2 + 1],
                            coef[:, CT2:CT2 + 1], Alu.mult)
            P.tensor_tensor(coef[:, CMPG:CMPG + 1], coef[:, CG:CG + 1],
                            mpi[:], Alu.mult)
            # q1 = -pi*G10*c30*cos(a14) ; q2 = 2*pi*G10*s30  (G10 = c2427^2)
            P.tensor_tensor(scr[:, 14:15], csc(Tt2427), csc(Tt2427), Alu.mult)
            P.tensor_tensor(coef[:, CT1:CT1 + 1], scr[:, 14:15], csc(Tt30),
                            Alu.mult)
            P.tensor_tensor(coef[:, CT2:CT2 + 1], coef[:, CT1:CT1 + 1],
                            csc(Xd14), Alu.mult)
            P.tensor_tensor(coef[:, CQ1:CQ1 + 1], coef[:, CT2:CT2 + 1],
                            mpi[:], Alu.mult)
            P.tensor_tensor(coef[:, CT1:CT1 + 1], scr[:, 14:15], css(Tt30),
                            Alu.mult)
            P.tensor_tensor(coef[:, CQ2:CQ2 + 1], coef[:, CT1:CT1 + 1],
                            p2T[:], Alu.mult)

            # ---------------- m9 merges (DVE) + m8 tan (Pool) -------------
            m4v = m4[:].rearrange("p (k b) -> p b k", b=2)
            P.tensor_tensor(m4v[:, 0], bc(csc(Xa11), (128, 2)),
                            cspair(Xa10p), Alu.mult)
            P.tensor_tensor(m4v[:, 1], bc(css(Xa11), (128, 2)),
                            swpair(Xa10m), Alu.mult)
            m8v = m8[:].rearrange("p (k b) -> p b k", b=2)
            P.tensor_tensor(m8v[:, 0], bc(csc(Xa13), (128, 4)), m4[:],
                            Alu.mult)
            m8hs = m8[:].rearrange("p (h l b) -> p b h l", h=2, l=2)
            m4hs = m4[:].rearrange("p (h l) -> p h l", h=2)[:, ::-1, :]
            P.tensor_tensor(m8hs[:, 1], bc(css(Xa13), (128, 2, 2)),
                            m4hs, Alu.mult)
            P.tensor_tensor(sqP[:, 0:4], m8[:, 4:8],
                            bc(cf(NT2427), (128, 4)), Alu.mult)
            P.tensor_tensor(m8n[:, 0:4], m8[:, 0:4], sqP[:, 0:4], Alu.add)
            P.tensor_tensor(sqP2[:, 0:4], m8[:, 0:4],
                            bc(cf(T2427), (128, 4)), Alu.mult)
            P.tensor_tensor(m8n[:, 4:8], m8[:, 4:8], sqP2[:, 0:4], Alu.add)


            pang = psmi.tile([128, 12], f32, tag="pang")
            # theta-row rank-1 updates start each accumulation region
            nc.tensor.matmul(pang[:, 0:12], ones1[:], THCr[:, 0:12],
                             start=True, stop=False, skip_group_check=True)
            for c in range(8):
                lst = (c == 7)
                nc.tensor.matmul(pang[:, 0:5], RiT[:, c * 128:(c + 1) * 128],
                                 bow[:, c * 10:c * 10 + 5],
                                 start=False, stop=False,
                                 skip_group_check=True)
                nc.tensor.matmul(pang[:, 5:8], RoT[:, c * 128:(c + 1) * 128],
                                 bow[:, c * 10 + 5:c * 10 + 8],
                                 start=False, stop=False,
                                 skip_group_check=True)
                nc.tensor.matmul(pang[:, 8:10], RiT[:, c * 128:(c + 1) * 128],
                                 bow[:, c * 10 + 1:c * 10 + 3],
                                 start=False, stop=False,
                                 skip_group_check=True)
                nc.tensor.matmul(pang[:, 10:12],
                                 RoT[:, c * 128:(c + 1) * 128],
                                 bow[:, c * 10 + 5:c * 10 + 7],
                                 start=False, stop=lst,
                                 skip_group_check=True)

            # ---------------- critical: pang -> ANG, sincos ---------------
            V.tensor_copy(ANG[:, sn(0):sn(0) + 12], pang[:])
            V.tensor_scalar(ANG[:, 0:12], pang[:], 1.0, 0.25,
                            Alu.mult, Alu.add)
            av = ANG[:].rearrange("p (a c) -> p a c", a=2)[:, :, 0:12]
            kiv = KI[:].rearrange("p (a c) -> p a c", a=2)[:, :, 0:12]
            kfv = KF[:].rearrange("p (a c) -> p a c", a=2)[:, :, 0:12]
            wv = WT[:].rearrange("p (a c) -> p a c", a=2)[:, :, 0:12]
            cv = CS[:].rearrange("p (a c) -> p a c", a=2)[:, :, 0:12]
            V.tensor_copy(kiv, av)
            V.tensor_copy(kfv, kiv)
            V.tensor_tensor(wv, av, kfv, Alu.subtract)
            S.activation(cv, wv, Act.Sin, scale=2.0 * PI)

            # ---------------- m9 finish (in the pang shadow) --------------
            S.activation(sq[:, 0:4], m8n[:, 0:4], Act.Square,
                         accum_out=zc(ZD8a))
            S.activation(sq[:, 4:8], m8n[:, 4:8], Act.Square,
                         accum_out=zc(ZD8b))
            V.scalar_tensor_tensor(sq2[:, 0:4], m8n[:, 0:4], 1.0,
                                   m8n[:, 4:8], Alu.mult, Alu.mult,
                                   accum_out=zc(ZC8))
            V.tensor_tensor(zc(ZD8), zc(ZD8a), zc(ZD8b), Alu.subtract)
            V.tensor_tensor(zc(ZT1), zc(ZD8), cf(CQ1), Alu.mult)
            V.scalar_tensor_tensor(zc(ZT2), zc(ZC8), cf(CQ2), zc(ZT1),
                                   Alu.mult, Alu.add)
            V.tensor_scalar(out_sb[:, 1:2], zc(ZT2), PI, None, Alu.add)
            nc.sync.dma_start(out_d[:, 1:2], out_sb[:, 1:2])


            # ---------------- critical tree -------------------------------
            # batched m0..m3 merge_cnots (2 wide DVE ops)
            MV = M03[:].rearrange("p (g1 g0 b1 b0) -> p b0 g1 g0 b1",
                                  g1=2, g0=2, b1=2, b0=2)
            LV = CS[:].rearrange("p (a u w) -> p a u w", a=2, u=7, w=4)
            CP = CS[:].rearrange("p (a u w) -> p u w a", a=2, u=7, w=4)
            CM = CS[:].rearrange("p (a m1 m0) -> p m1 m0 a", a=2, m1=14, m0=2)
            Lcos = LV[:, 0, 0:2, 0:4:3].unsqueeze(3).to_broadcast(
                (128, 2, 2, 2))
            Lsin = LV[:, 1, 0:2, 0:4:3].unsqueeze(3).to_broadcast(
                (128, 2, 2, 2))
            V.tensor_tensor(MV[:, 0], Lcos, CP[:, 0:2, 1:3, :], Alu.mult)
            V.tensor_tensor(MV[:, 1], Lsin, CM[:, 4:6, :, ::-1], Alu.mult)
            # m0 tan(th25) on bit0: pairs {0,2}/{1,3} of M03[:,0:4]
            m0v = M03[:, 0:4].rearrange("p (k b) -> p b k", b=2)
            m0nv = m0n[:].rearrange("p (k b) -> p b k", b=2)
            V.scalar_tensor_tensor(m0nv[:, 0], m0v[:, 1], cf(NT25),
                                   m0v[:, 0], Alu.mult, Alu.add)
            V.scalar_tensor_tensor(m0nv[:, 1], m0v[:, 0], cf(T25),
                                   m0v[:, 1], Alu.mult, Alu.add)

            def merge_cnot2(dst, L, H, eng):
                dv = dst[:].rearrange("p (h l k) -> p h l k", h=2, l=2)
                Lv = L.rearrange("p (b k) -> p b k", b=2)
                Hv = H.rearrange("p (h l) -> p h l", h=2)
                Hn = Hv.unsqueeze(3).to_broadcast((128, 2, 2, 2))
                Hs = Hv[:, ::-1, :].unsqueeze(3).to_broadcast((128, 2, 2, 2))
                L0 = Lv[:, 0].rearrange("p (o k) -> p o k", o=1)
                L0 = L0.unsqueeze(1).to_broadcast((128, 2, 2, 2))
                L1 = Lv[:, 1].rearrange("p (o k) -> p o k", o=1)
                L1 = L1.unsqueeze(1).to_broadcast((128, 2, 2, 2))
                eng.tensor_tensor(dv[:, :, :, 0:2].squeeze(), L0.squeeze(),
                                  Hn.squeeze(), Alu.mult)
                eng.tensor_tensor(dv[:, :, :, 2:4].squeeze(), L1.squeeze(),
                                  Hs.squeeze(), Alu.mult)

            # Pool lane: m6 = mc(m3, m2) + tan(th20), u/w split
            merge_cnot2(m6, M03[:, 12:16], M03[:, 8:12], P)
            P.tensor_tensor(sqP[:, 0:8], m6[:, 8:16],
                            bc(cf(NT20), (128, 8)), Alu.mult)
            P.tensor_tensor(m6n[:, 0:8], m6[:, 0:8], sqP[:, 0:8], Alu.add)
            P.tensor_tensor(sqP2[:, 0:8], m6[:, 0:8],
                            bc(cf(T20), (128, 8)), Alu.mult)
            P.tensor_tensor(m6n[:, 8:16], m6[:, 8:16], sqP2[:, 0:8], Alu.add)
            P.tensor_tensor(uw6[:, 0:8], m6n[:, 0:8], m6n[:, 8:16],
                            Alu.subtract)
            P.tensor_tensor(uw6[:, 8:16], m6n[:, 0:8], m6n[:, 8:16],
                            Alu.add)

            # DVE lane: m5 = mc(m0n, m1) + tan(th19) (+ negated hi half)
            merge_cnot2(m5, m0n[:], M03[:, 4:8], V)
            V.scalar_tensor_tensor(m5n[:, 0:8], m5[:, 8:16], cf(NT19),
                                   m5[:, 0:8], Alu.mult, Alu.add)
            V.scalar_tensor_tensor(m5n[:, 8:16], m5[:, 0:8], cf(T19),
                                   m5[:, 8:16], Alu.mult, Alu.add)
            V.scalar_tensor_tensor(m5n[:, 16:24], m5[:, 0:8], cf(NT19),
                                   m5[:, 8:16], Alu.mult, Alu.subtract)

            # A/B accumulations (bit0-masked, signs/scales folded)
            ev = m5n[:, 0:16:2]                     # b0=0, 8 elems
            ov = m5n[:, 1:16:2]                     # b0=1
            m5h = m5n[:].rearrange("p (h j) -> p h j", h=3)
            ev2 = m5n[:, 0:16].rearrange("p (h j) -> p h j", h=2)[:, :, 0:8:2]
            ov2 = m5n[:, 0:16].rearrange("p (h j) -> p h j", h=2)[:, :, 1:8:2]
            pme = m5h[:, 0:3:2, 0:8:2]
            pmo = m5h[:, 0:3:2, 1:8:2]
            sqv = sq[:, 0:8].rearrange("p (h j) -> p h j", h=2)
            sqv2 = sq2[:, 0:8].rearrange("p (h j) -> p h j", h=2)
            V.scalar_tensor_tensor(sq[:, 0:8], ev, -2.0, ev,
                                   Alu.mult, Alu.mult, accum_out=zc(ZA))
            V.scalar_tensor_tensor(sq2[:, 0:8], ov, -2.0, ov,
                                   Alu.mult, Alu.mult, accum_out=zc(ZA + 1))
            V.scalar_tensor_tensor(sqv, ev2, 1.0, pme,
                                   Alu.mult, Alu.mult, accum_out=zc(ZB))
            V.scalar_tensor_tensor(sqv2, ov2, -1.0, pmo,
                                   Alu.mult, Alu.mult, accum_out=zc(ZB + 1))

            # wB/uB: D6-side coefficient from cos(Tsum), cos(Tdif)
            V.tensor_tensor(zacc[:, ZWB:ZWB + 2], zacc[:, ZB:ZB + 2],
                            CS[:, Tsum:Tsum + 2], Alu.mult)
            V.tensor_tensor(zc(ZUB), zacc[:, ZWB:ZWB + 1],
                            zacc[:, ZWB + 1:ZWB + 2], Alu.add)
            # wA/vA: C6-side from sin(Tsum), sin(Tdif); -2 already in A
            V.tensor_tensor(zacc[:, ZWA:ZWA + 2], zacc[:, ZA:ZA + 2],
                            CS[:, NA + Tsum:NA + Tsum + 2], Alu.mult)
            V.tensor_tensor(zc(ZVA), zacc[:, ZWA:ZWA + 1],
                            zacc[:, ZWA + 1:ZWA + 2], Alu.add)

            # D6/C6 and final combine
            V.scalar_tensor_tensor(sq[:, 0:8], uw6[:, 0:8], 1.0,
                                   uw6[:, 8:16], Alu.mult, Alu.mult,
                                   accum_out=zc(ZD6))
            V.scalar_tensor_tensor(sq2[:, 0:8], m6n[:, 0:8], 1.0,
                                   m6n[:, 8:16], Alu.mult, Alu.mult,
                                   accum_out=zc(ZC6))
            V.tensor_tensor(zc(ZZD), zc(ZD6), zc(ZUB), Alu.mult)
            V.scalar_tensor_tensor(zc(ZZ5), zc(ZC6), zacc[:, ZVA:ZVA + 1],
                                   zc(ZZD), Alu.mult, Alu.add)
            V.scalar_tensor_tensor(out_sb[:, 0:1], zc(ZZ5), cf(CMPG),
                                   piT[:], Alu.mult, Alu.add)
            nc.sync.dma_start(out_d[:, 0:1], out_sb[:, 0:1])

    nc.compile()
    return nc


def get_nc():
    if "nc" not in _cache:
        _cache["nc"] = _build_program()
    return _cache["nc"]


def kernel(X, e, Ri, Ro, theta):
    import ml_dtypes
    from concourse.bass_utils import run_bass_kernel_spmd

    nc = get_nc()
    bf16 = ml_dtypes.bfloat16
    X = np.asarray(X, dtype=np.float32)
    e = np.asarray(e, dtype=np.float32)
    Ri = np.asarray(Ri, dtype=np.float32)
    Ro = np.asarray(Ro, dtype=np.float32)
    theta = np.asarray(theta, dtype=np.float32)
    e8 = np.ascontiguousarray(e.reshape(8, 128).T)
    thB = np.broadcast_to(theta, (128, 31))
    P1 = np.ascontiguousarray(
        np.concatenate([X, e8, thB], axis=1), dtype=np.float32)
    P2 = np.ascontiguousarray(
        np.concatenate([X, e8], axis=1)).astype(bf16)
    in_map = {
        "Rib": Ri.astype(bf16),
        "Rob": Ro.astype(bf16),
        "P1": P1,
        "P2": P2,
    }
    res = run_bass_kernel_spmd(
        nc, [dict(in_map) for _ in range(N_CORES)],
        core_ids=list(range(N_CORES)),
    )
    return res.results[0]["out"]
